# revision 4
# baseline (speedup 1.0000x reference)
"""Trainium2 Bass kernel for the AdaptiveFF spiking network.

Sharding: data-parallel over batch, 8 NeuronCores, 32 batch elements per
core, weights replicated. No collectives needed.

Per-core kernel structure (all state feature-major: [feature, batch] with
feature chunks of 128 on SBUF partitions):
  - The input-layer matmul X = inp @ W1.T is hoisted out of the sim loop
    (x_t is constant across the 4 sim steps) and batched over t-blocks.
  - The v1/s1 LIF chain depends only on X, so the spikes of a whole
    t-block are computed first; the W2/W3 matmuls then batch over
    (t, sim_step) with N=512 free-dim tiles.
  - The output integrator is linear, so the W4 matmul runs once per t on
    z = sum_k beta^(3-k) s2_k instead of once per sim step.
  - W2/W3 run as two matmul passes each: bf16 high part + fp16 low
    residual, reproducing the fp32 product to ~1e-8 at 2x the fp32 rate.
    W1/W4 matmuls are fp32.
Emission is software-pipelined with a 2-round skew so the DVE state
chains, PE matmuls and ACT psum-evictions of adjacent blocks overlap.
"""

import sys

for p in ("/opt/trn_rl_repo", "/root/.axon_site/_ro/trn_rl_repo"):
    if p not in sys.path:
        sys.path.append(p)

from contextlib import ExitStack

import numpy as np
import ml_dtypes

from concourse import mybir
import concourse.bass as bass
import concourse.tile as tile
from concourse.tile import TileContext
from concourse.bass_utils import run_bass_kernel_spmd

F32 = mybir.dt.float32
BF16 = mybir.dt.bfloat16
F16 = mybir.dt.float16
ALU = mybir.AluOpType
ACTF = mybir.ActivationFunctionType

T, B, NIN = 200, 256, 700
NS1, NA, NS2, NOUT = 512, 256, 512, 20
SIM = 4
BETA, THRESH, BETA_B, RHO = 0.9, 1.0, 0.95, 0.5
NCORES = 8
BC = B // NCORES          # 32 batch per core
TB = 8                    # time-block
NBLK = T // TB
NC1 = NS1 // 128          # 4 feature chunks for s1/x2
NC2 = NA // 128           # 2 chunks for sa/ax
NCI = (NIN + 127) // 128  # 6 input chunks (last ragged: 60)
NB = TB * BC              # 256 (t, b) cols per block
NKB = TB * SIM * BC       # 1024 (t, k, b) cols per block

_CACHE = {}


def _split_waits(nc, max_waits=1):
    """walrus in this container rejects >1 sem-wait per instruction; hoist
    extras onto preceding InstEventSemaphore instructions on the same
    engine (program order makes them happen-before)."""
    for f in nc.m.functions:
        for bb in f.blocks:
            dirty = False
            newl = []
            for ins in bb.instructions:
                si = ins.sync_info
                if si is not None and len(si.on_wait) > max_waits:
                    waits = list(si.on_wait)
                    for w in waits[:-max_waits]:
                        ev = mybir.InstEventSemaphore(
                            name=nc.get_next_instruction_name(), ins=[], outs=[])
                        ev.engine = ins.engine
                        ev.sync_info = mybir.SyncInfo(on_wait=[w], on_update=[])
                        nc.register_instruction(ev, overwrite=True)
                        newl.append(ev)
                    ins.sync_info = mybir.SyncInfo(
                        on_wait=waits[-max_waits:], on_update=list(si.on_update))
                    dirty = True
                newl.append(ins)
            if dirty:
                bb.instructions = newl


def _patch_tile_drain():
    if getattr(tile.TileContext, "_wait_split_patched", False):
        return
    orig = tile.TileContext._drain_and_barrier

    def patched(self, tick_clock, wait_clock):
        orig(self, tick_clock, wait_clock)
        _split_waits(self.nc)

    tile.TileContext._drain_and_barrier = patched
    tile.TileContext._wait_split_patched = True


def build_nc():
    _patch_tile_drain()
    nc = bass.Bass("TRN2", target_bir_lowering=False)

    dp = nc.declare_dram_parameter
    inpT = dp("inpT", [NIN, T, BC], F32, isOutput=False)
    w1t = dp("w1t", [NIN, NS1], F32, isOutput=False)
    w2h = dp("w2h", [NS1, NA], BF16, isOutput=False)
    w2l = dp("w2l", [NS1, NA], F16, isOutput=False)
    w3h = dp("w3h", [NS1 + NA, NS2], BF16, isOutput=False)
    w3l = dp("w3l", [NS1 + NA, NS2], F16, isOutput=False)
    w4t = dp("w4t", [NS2, NOUT], F32, isOutput=False)
    b1m = dp("b1m", [NC1, 128], F32, isOutput=False)
    b2m = dp("b2m", [NC2, 128], F32, isOutput=False)
    b3m = dp("b3m", [NC1, 128], F32, isOutput=False)
    b4c = dp("b4c", [NOUT, 1], F32, isOutput=False)
    outT = dp("outT", [NOUT, T, BC], F32, isOutput=True)

    with TileContext(nc) as tc, ExitStack() as ctx:
        wpool = ctx.enter_context(tc.tile_pool(name="weights", bufs=1))
        spool = ctx.enter_context(tc.tile_pool(name="states", bufs=1))
        xpool = ctx.enter_context(tc.tile_pool(name="xbuf", bufs=2))
        s1pool = ctx.enter_context(tc.tile_pool(name="s1buf", bufs=2))
        sapool = ctx.enter_context(tc.tile_pool(name="sabuf", bufs=2))
        axpool = ctx.enter_context(tc.tile_pool(name="axbuf", bufs=2))
        x2pool = ctx.enter_context(tc.tile_pool(name="x2buf", bufs=2))
        zpool = ctx.enter_context(tc.tile_pool(name="zbuf", bufs=2))
        ipool = ctx.enter_context(tc.tile_pool(name="inp", bufs=2))
        opool = ctx.enter_context(tc.tile_pool(name="outt", bufs=2))
        pxpool = ctx.enter_context(tc.tile_pool(name="px", bufs=2, space="PSUM"))
        pmpool = ctx.enter_context(tc.tile_pool(name="pmid", bufs=4, space="PSUM"))
        popool = ctx.enter_context(tc.tile_pool(name="po", bufs=2, space="PSUM"))

        # ---- load weights ----
        w1 = []
        for c in range(NCI):
            kc = min(128, NIN - c * 128)
            wt = wpool.tile([kc, NS1], F32, tag=f"w1_{c}", name=f"w1_{c}")
            nc.sync.dma_start(out=wt[:], in_=w1t[c * 128:c * 128 + kc, :])
            w1.append(wt)
        w2hp, w2lp, w3hp, w3lp, w4p = [], [], [], [], []
        for c in range(NC1):
            wt = wpool.tile([128, NA], BF16, tag=f"w2h_{c}", name=f"w2h_{c}")
            nc.sync.dma_start(out=wt[:], in_=w2h[c * 128:(c + 1) * 128, :])
            w2hp.append(wt)
            wt = wpool.tile([128, NA], F16, tag=f"w2l_{c}", name=f"w2l_{c}")
            nc.sync.dma_start(out=wt[:], in_=w2l[c * 128:(c + 1) * 128, :])
            w2lp.append(wt)
        for c in range(NC1 + NC2):
            wt = wpool.tile([128, NS2], BF16, tag=f"w3h_{c}", name=f"w3h_{c}")
            nc.sync.dma_start(out=wt[:], in_=w3h[c * 128:(c + 1) * 128, :])
            w3hp.append(wt)
            wt = wpool.tile([128, NS2], F16, tag=f"w3l_{c}", name=f"w3l_{c}")
            nc.sync.dma_start(out=wt[:], in_=w3l[c * 128:(c + 1) * 128, :])
            w3lp.append(wt)
        for c in range(NC1):
            wt = wpool.tile([128, NOUT], F32, tag=f"w4_{c}", name=f"w4_{c}")
            nc.sync.dma_start(out=wt[:], in_=w4t[c * 128:(c + 1) * 128, :])
            w4p.append(wt)
        b1t = wpool.tile([128, NC1], F32, tag="b1t", name="b1t")
        nc.sync.dma_start(out=b1t[:], in_=bass.AP(b1m, 0, [[1, 128], [128, NC1]]))
        b2t = wpool.tile([128, NC2], F32, tag="b2t", name="b2t")
        nc.sync.dma_start(out=b2t[:], in_=bass.AP(b2m, 0, [[1, 128], [128, NC2]]))
        b3t = wpool.tile([128, NC1], F32, tag="b3t", name="b3t")
        nc.sync.dma_start(out=b3t[:], in_=bass.AP(b3m, 0, [[1, 128], [128, NC1]]))
        b4t = wpool.tile([NOUT, 1], F32, tag="b4t", name="b4t")
        nc.sync.dma_start(out=b4t[:], in_=b4c[:, :])

        # ---- persistent states, layout [128, chunk*BC + b] ----
        v1 = spool.tile([128, NC1 * BC], F32, tag="v1", name="v1")
        va = spool.tile([128, NC2 * BC], F32, tag="va", name="va")
        ba = spool.tile([128, NC2 * BC], F32, tag="ba", name="ba")
        thr = spool.tile([128, NC2 * BC], F32, tag="thr", name="thr")
        sth = spool.tile([128, NC2 * BC], F32, tag="sth", name="sth")
        v2 = spool.tile([128, NC1 * BC], F32, tag="v2", name="v2")
        zacc = spool.tile([128, NC1 * BC], F32, tag="zacc", name="zacc")
        s2s = spool.tile([128, NC1 * BC], F32, tag="s2s", name="s2s")
        for st in (v1, va, ba, v2):
            nc.vector.memset(st[:], 0.0)

        # per-block tiles carried between skewed emission rounds
        S1 = [None] * NBLK
        SA = [None] * NBLK
        AXB = [None] * NBLK
        X2B = [None] * NBLK

        def emit_front(i):
            """inp DMA, L1 matmuls, X eviction, v1/s1 chain, L2 matmuls."""
            t0 = i * TB
            itiles = []
            for c in range(NCI):
                kc = min(128, NIN - c * 128)
                it = ipool.tile([kc, NB], F32, tag=f"inp_{c}", name=f"inp_{c}")
                nc.sync.dma_start(
                    out=it[:],
                    in_=bass.AP(inpT, c * 128 * T * BC + t0 * BC,
                                [[T * BC, kc], [1, NB]]))
                itiles.append(it)
            # L1: X.T [512, (t,b)] into two psum banks [128, 2*NB]
            px = [pxpool.tile([128, 2 * NB], F32, tag="px", name="px") for _ in range(2)]
            for mt in range(2):
                for m2 in range(2):
                    m = 2 * mt + m2
                    for c in range(NCI):
                        nc.tensor.matmul(
                            px[mt][:, m2 * NB:(m2 + 1) * NB],
                            w1[c][:, m * 128:(m + 1) * 128],
                            itiles[c][:],
                            start=(c == 0), stop=(c == NCI - 1))
            X = xpool.tile([128, NC1 * NB], F32, tag="X", name="X")
            for m in range(NC1):
                nc.scalar.activation(
                    X[:, m * NB:(m + 1) * NB],
                    px[m // 2][:, (m % 2) * NB:(m % 2 + 1) * NB],
                    ACTF.Identity, bias=b1t[:, m:m + 1])
            # v1/s1 chain; spikes into S1 [128, (c, t, k, b)]
            s1 = s1pool.tile([128, NC1 * NKB], BF16, tag="S1", name="S1")
            S1[i] = s1
            Xr = X.rearrange("p (m t b) -> p m t b", m=NC1, t=TB)
            s1r = s1.rearrange("p (c t k b) -> p c t k b", c=NC1, t=TB, k=SIM)
            for tt in range(TB):
                xap = Xr[:, :, tt, :]
                for k in range(SIM):
                    nc.vector.scalar_tensor_tensor(
                        v1[:], v1[:], BETA, xap, ALU.mult, ALU.add)
                    sap = s1r[:, :, tt, k, :]
                    nc.vector.tensor_scalar(sap, v1[:], THRESH, None, ALU.is_gt)
                    nc.vector.tensor_tensor(v1[:], v1[:], sap, ALU.subtract)
            # L2: ax.T [256, (t,k,b)] -> AX sbuf with +b2
            ax = axpool.tile([128, NC2 * NKB], F32, tag="AX", name="AX")
            AXB[i] = ax
            for m2 in range(NC2):
                for n in range(NKB // 512):
                    pm = pmpool.tile([128, 512], F32, tag="pm", name="pm")
                    for ph, wp in enumerate((w2hp, w2lp)):
                        for c in range(NC1):
                            nc.tensor.matmul(
                                pm[:],
                                wp[c][:, m2 * 128:(m2 + 1) * 128],
                                s1[:, c * NKB + n * 512:c * NKB + (n + 1) * 512],
                                start=(ph == 0 and c == 0),
                                stop=(ph == 1 and c == NC1 - 1))
                    nc.scalar.activation(
                        ax[:, m2 * NKB + n * 512:m2 * NKB + (n + 1) * 512],
                        pm[:], ACTF.Identity, bias=b2t[:, m2:m2 + 1])

        def emit_mid(i):
            """ALIF chain for block i, then L3 matmuls -> X2 eviction."""
            ax = AXB[i]
            sa = sapool.tile([128, NC2 * NKB], BF16, tag="SA", name="SA")
            SA[i] = sa
            axr = ax.rearrange("p (c t k b) -> p c t k b", c=NC2, t=TB, k=SIM)
            sar = sa.rearrange("p (c t k b) -> p c t k b", c=NC2, t=TB, k=SIM)
            for tt in range(TB):
                for k in range(SIM):
                    axap = axr[:, :, tt, k, :]
                    saap = sar[:, :, tt, k, :]
                    nc.vector.scalar_tensor_tensor(
                        va[:], va[:], BETA, axap, ALU.mult, ALU.add)
                    nc.vector.tensor_scalar(
                        thr[:], ba[:], RHO, THRESH, ALU.mult, ALU.add)
                    nc.vector.tensor_tensor(saap, va[:], thr[:], ALU.is_gt)
                    nc.vector.tensor_tensor(sth[:], saap, thr[:], ALU.mult)
                    nc.vector.tensor_tensor(va[:], va[:], sth[:], ALU.subtract)
                    nc.vector.scalar_tensor_tensor(
                        ba[:], ba[:], BETA_B, saap, ALU.mult, ALU.add)
            # L3: x2.T [512, (t,k,b)] -> X2 sbuf with +b3
            s1 = S1[i]
            x2 = x2pool.tile([128, NC1 * NKB], F32, tag="X2", name="X2")
            X2B[i] = x2
            for m in range(NC1):
                for n in range(NKB // 512):
                    pm = pmpool.tile([128, 512], F32, tag="pm", name="pm")
                    for ph, wp in enumerate((w3hp, w3lp)):
                        for c in range(NC1):
                            nc.tensor.matmul(
                                pm[:],
                                wp[c][:, m * 128:(m + 1) * 128],
                                s1[:, c * NKB + n * 512:c * NKB + (n + 1) * 512],
                                start=(ph == 0 and c == 0), stop=False)
                        for c2 in range(NC2):
                            nc.tensor.matmul(
                                pm[:],
                                wp[NC1 + c2][:, m * 128:(m + 1) * 128],
                                sa[:, c2 * NKB + n * 512:c2 * NKB + (n + 1) * 512],
                                start=False,
                                stop=(ph == 1 and c2 == NC2 - 1))
                    nc.scalar.activation(
                        x2[:, m * NKB + n * 512:m * NKB + (n + 1) * 512],
                        pm[:], ACTF.Identity, bias=b3t[:, m:m + 1])

        def emit_back(i):
            """v2/s2 chain + z integration for block i, then L4 + out DMA."""
            t0 = i * TB
            x2 = X2B[i]
            x2r = x2.rearrange("p (c t k b) -> p c t k b", c=NC1, t=TB, k=SIM)
            zb = zpool.tile([128, TB * NC1 * BC], F32, tag="ZB", name="ZB")
            zbr = zb.rearrange("p (t m b) -> p t m b", t=TB, m=NC1)
            for tt in range(TB):
                for k in range(SIM):
                    nc.vector.scalar_tensor_tensor(
                        v2[:], v2[:], BETA, x2r[:, :, tt, k, :], ALU.mult, ALU.add)
                    if k == 0:
                        # z := s2 at the start of each t's integrator
                        nc.vector.tensor_scalar(
                            zacc[:], v2[:], THRESH, None, ALU.is_gt)
                        nc.vector.tensor_tensor(v2[:], v2[:], zacc[:], ALU.subtract)
                    else:
                        nc.vector.tensor_scalar(
                            s2s[:], v2[:], THRESH, None, ALU.is_gt)
                        nc.vector.tensor_tensor(v2[:], v2[:], s2s[:], ALU.subtract)
                        zdst = zb[:, tt * NC1 * BC:(tt + 1) * NC1 * BC] \
                            if k == SIM - 1 else zacc[:]
                        nc.vector.scalar_tensor_tensor(
                            zdst, zacc[:], BETA, s2s[:], ALU.mult, ALU.add)
            # L4: o.T [20, (t,b)] fp32
            po = popool.tile([NOUT, NB], F32, tag="po", name="po")
            for c in range(NC1):
                nc.tensor.matmul(
                    po[:], w4p[c][:], zbr[:, :, c, :],
                    start=(c == 0), stop=(c == NC1 - 1))
            ot = opool.tile([NOUT, NB], F32, tag="OT", name="OT")
            nc.scalar.activation(ot[:], po[:], ACTF.Identity, bias=b4t[:, 0:1])
            nc.sync.dma_start(
                out=bass.AP(outT, t0 * BC, [[T * BC, NOUT], [1, NB]]),
                in_=ot[:])

        # software-pipelined emission with 2-round skew
        for r in range(NBLK + 2):
            if r < NBLK:
                emit_front(r)
            if 1 <= r < NBLK + 1:
                emit_mid(r - 1)
            if r >= 2:
                emit_back(r - 2)

    return nc


def _prep_host(inputs):
    inp = np.ascontiguousarray(inputs["inp"], dtype=np.float32)
    W1 = np.asarray(inputs["W1"], np.float32)
    W2 = np.asarray(inputs["W2"], np.float32)
    W3 = np.asarray(inputs["W3"], np.float32)
    W4 = np.asarray(inputs["W4"], np.float32)
    b1 = np.asarray(inputs["b1"], np.float32)
    b2 = np.asarray(inputs["b2"], np.float32)
    b3 = np.asarray(inputs["b3"], np.float32)
    b4 = np.asarray(inputs["b4"], np.float32)

    def split(W):
        Wh = W.T.astype(ml_dtypes.bfloat16)
        Wl = (W.T - Wh.astype(np.float32)).astype(np.float16)
        return np.ascontiguousarray(Wh), np.ascontiguousarray(Wl)

    w2h, w2l = split(W2)
    w3h, w3l = split(W3)
    csum = float(sum(BETA ** k for k in range(SIM)))
    shared = dict(
        w1t=np.ascontiguousarray(W1.T),
        w2h=w2h, w2l=w2l, w3h=w3h, w3l=w3l,
        w4t=np.ascontiguousarray(W4.T),
        b1m=np.ascontiguousarray(b1.reshape(NC1, 128)),
        b2m=np.ascontiguousarray(b2.reshape(NC2, 128)),
        b3m=np.ascontiguousarray(b3.reshape(NC1, 128)),
        b4c=np.ascontiguousarray((b4.astype(np.float64) * csum)
                                 .astype(np.float32).reshape(NOUT, 1)),
    )
    in_maps = []
    for c in range(NCORES):
        shard = inp[:, c * BC:(c + 1) * BC, :]                 # [T, BC, NIN]
        m = dict(shared)
        m["inpT"] = np.ascontiguousarray(shard.transpose(2, 0, 1))
        in_maps.append(m)
    return in_maps


def run(inputs, trace=False, **kw):
    if "nc" not in _CACHE:
        _CACHE["nc"] = build_nc()
    nc = _CACHE["nc"]
    in_maps = _prep_host(inputs)
    res = run_bass_kernel_spmd(nc, in_maps, core_ids=list(range(NCORES)),
                               trace=trace, **kw)
    outs = []
    for c in range(NCORES):
        outT = res.results[c]["outT"]                          # [NOUT, T, BC]
        outs.append(np.ascontiguousarray(outT.transpose(1, 2, 0)))
    full = np.concatenate(outs, axis=1)                        # [T, B, NOUT]
    return full, res


def kernel(**inputs):
    out, _ = run(inputs)
    return out
